# revision 55
# baseline (speedup 1.0000x reference)
"""Trainium2 Bass kernel for nn_PrimalDual (primal-dual multi-label segmentation).

Strategy (v2):
  - Shard image rows (h) across 8 cores; ROWS=48 owned + G=repeats ghost rows
    each side computed redundantly (ghost shrinks 1 row/iter; no comms).
  - Layout: partition q in [0,128) holds image columns w = C*q + c; free dims
    (h_local, c, z|proj). All state SBUF-resident, f16 (u too).
  - Dual state is stored tau-scaled (s~ = tau*s, mu~ = tau*mu) and the PROJ
    axis is enumerated k2-major, so interval sums and the mu->z sums are
    forward segmented scans plus contiguous slice ops:
        t~ = intervalsum(tau*p)          (z-cumsum + 12 slice ops)
        mu~' = mu~ + (s~ - t~);  m~ = t~ - mu~'   (identity: m = s - mu - 2dl)
        msum~ = segscan(mu~') diag-accumulated over k2 runs
  - No DVE reciprocals: divisions/powers go through ACT Ln/Exp; ACT ops are
    batched per block in table-set order (sqrt -> ln/exp -> trig) to minimize
    1283ns activation-table loads.
  - Masks are uint16 (2-byte keeps DVE 2x modes; valid for copy_predicated).
  - Pool (gpsimd) engine takes a slice of the PROJ-sized adds.
"""

import numpy as np
from contextlib import ExitStack

import concourse.bass as bass
import concourse.tile as tile
from concourse import bacc, mybir
from concourse.bass_utils import run_bass_kernel_spmd

# Force the act-table pass to pick the combined ln+exp set: strip Ln/Exp from
# every other set (order/ids preserved, so walrus still sees valid indices).
_orig_get_tables = bacc.get_activation_tables


def _patched_get_tables(arch):
    tabs = {k: set(v) for k, v in _orig_get_tables(arch).items()}
    comb = "natural_log_exp_and_others"
    if comb in tabs:
        ln = mybir.ActivationFunctionType.Ln
        ex = mybir.ActivationFunctionType.Exp
        for name, fns in tabs.items():
            if name != comb:
                fns.discard(ln)
                fns.discard(ex)
    return tabs


bacc.get_activation_tables = _patched_get_tables

F16 = mybir.dt.float16
U16 = mybir.dt.uint16
F32 = mybir.dt.float32
AF = mybir.ActivationFunctionType
OP = mybir.AluOpType

CFG = dict(H=384, W=384, L=12, NCORES=8, P=128)

AB = 8    # A/C-phase row-block
BB = 17   # B-phase row-block

_HALF_PI = 1.5707963267948966


def flat(ap):
    nd = len(ap.shape)
    if nd == 2:
        return ap
    names = " ".join(f"d{i}" for i in range(nd - 1))
    return ap.rearrange(f"p {names} -> p ({names})")


def _register_consts(nc, values):
    for v in values:
        v = float(v)
        if (mybir.dt.float32, v) in nc.const_aps.aps:
            continue
        t = nc.alloc_sbuf_tensor(f"constf32-{len(nc.const_aps.aps)}", [128, 1], F32)
        nc.gpsimd.memset(t.ap(), v)
        nc.const_aps.aps[(mybir.dt.float32, v)] = t.ap()
    nc.all_engine_barrier()


def _blocks(lo, hi, step):
    out = []
    r = lo
    while r < hi:
        out.append((r, min(r + step, hi)))
        r = out[-1][1]
    return out


def build_program(lmbda, nu, repeats, l, cfg=None):
    cfg = cfg or CFG
    H, W, L, NCORES, P = cfg["H"], cfg["W"], cfg["L"], cfg["NCORES"], cfg["P"]
    assert L == l
    assert W % P == 0
    C = W // P
    ROWS = H // NCORES
    G = repeats
    SLAB = ROWS + 2 * G
    PROJ = l * (l + 1) // 2

    sigmap = 1.0 / (3.0 + l)
    tauu = 1.0 / 6.0
    tau_mu = 1.0 / (2.0 + PROJ / 4.0)
    lmbda = float(lmbda)
    nu = float(nu)
    sql = float(np.sqrt(lmbda))
    kl = [(z + 1) / l for z in range(l)]
    ln_nu = float(np.log(nu))
    ln_half = float(np.log(0.5))
    ln_two = float(np.log(2.0))

    # k2-major run offsets: run k2 holds k1 = 0..k2, length k2+1
    off2 = [k2 * (k2 + 1) // 2 for k2 in range(l + 1)]

    nc = bacc.Bacc("TRN2", target_bir_lowering=False, debug=False,
                   num_devices=NCORES)
    _register_consts(nc, [sql * k for k in kl] +
                     [_HALF_PI, ln_nu, ln_half, ln_two, 0.0])

    f_in = nc.dram_tensor("f_in", [P, SLAB * C], F32, kind="ExternalInput")
    mA_in = nc.dram_tensor("mA_in", [P, SLAB], F16, kind="ExternalInput")
    mC_in = nc.dram_tensor("mC_in", [P, SLAB], F16, kind="ExternalInput")
    wm_in = nc.dram_tensor("wm_in", [P, 2], F32, kind="ExternalInput")
    u_out = nc.dram_tensor("u_out", [P, ROWS * C * L], F16, kind="ExternalOutput")

    with tile.TileContext(nc) as tc, ExitStack() as ctx, \
            nc.allow_low_precision(reason="f16 state by design"):
        V = nc.vector
        S = nc.scalar
        G_ = nc.gpsimd

        st = ctx.enter_context(tc.tile_pool(name="state", bufs=1))
        u = st.tile([P, SLAB, C, L], F16)
        ubar = st.tile([P, SLAB, C, L], F16)
        p1 = st.tile([P, SLAB, C, L], F16)
        p2 = st.tile([P, SLAB, C, L], F16)
        p3 = st.tile([P, SLAB, C, L], F16)
        s1 = st.tile([P, SLAB, C, PROJ], F16)
        s2 = st.tile([P, SLAB, C, PROJ], F16)
        mu1 = st.tile([P, SLAB, C, PROJ], F16)
        mu2 = st.tile([P, SLAB, C, PROJ], F16)
        ld2 = st.tile([P, SLAB, C, L], F16)
        msum1 = st.tile([P, SLAB, C, L], F16)   # tau-scaled mu->z sums
        msum2 = st.tile([P, SLAB, C, L], F16)
        mAx = st.tile([P, SLAB, C, L], F16)     # expanded edge masks
        mCx = st.tile([P, SLAB, C, L], F16)
        fsb = st.tile([P, SLAB, C], F32)
        zmb13 = st.tile([P, BB, C, 13], F16)    # z-scan mask (0 at col 0)
        pmb2 = st.tile([P, BB, C, PROJ], F16)   # proj-scan mask, k2-major
        wm = st.tile([P, 2], F32)
        wsu = st.tile([P, SLAB, L], F16)
        wsp = st.tile([P, SLAB, L], F16)

        at_ = ctx.enter_context(tc.tile_pool(name="atemp", bufs=2))
        bt_ = ctx.enter_context(tc.tile_pool(name="btemp", bufs=1))

        def atile(tag, dt=F16):
            return at_.tile([P, AB, C, L], dt, tag=tag, name=tag)

        def btile(tag, dt=F16):
            return bt_.tile([P, BB, C, PROJ], dt, tag=tag, name=tag)

        # ---------------- init ----------------
        nc.sync.dma_start(flat(fsb[:]), f_in.ap())
        nc.sync.dma_start(wm[:], wm_in.ap())
        fb = fsb[:].unsqueeze(3).broadcast_to([P, SLAB, C, L])
        V.tensor_copy(u[:], fb)
        S.activation(ubar[:, 0:12], fsb[:, 0:12].unsqueeze(3)
                     .broadcast_to([P, 12, C, L]), AF.Copy)
        S.activation(ubar[:, 12:SLAB], fsb[:, 12:SLAB].unsqueeze(3)
                     .broadcast_to([P, SLAB - 12, C, L]), AF.Copy)
        for z in range(L):
            S.activation(ld2[:, :, :, z:z + 1], fsb[:].unsqueeze(3),
                         AF.Square, scale=-sql, bias=sql * kl[z])
        for t in (p1, p2, p3, s1, s2, mu1, mu2, msum1, msum2):
            G_.memset(t[:], 0.0)
        # expanded edge masks (reuse fsb staging buffer for the DMA'd rows)
        mArow = st.tile([P, SLAB], F16)
        mCrow = st.tile([P, SLAB], F16)
        nc.sync.dma_start(mArow[:], mA_in.ap())
        nc.sync.dma_start(mCrow[:], mC_in.ap())
        S.activation(mAx[:], mArow[:].unsqueeze(2).unsqueeze(3)
                     .broadcast_to([P, SLAB, C, L]), AF.Copy)
        S.activation(mCx[:], mCrow[:].unsqueeze(2).unsqueeze(3)
                     .broadcast_to([P, SLAB, C, L]), AF.Copy)
        V.memset(zmb13[:], 1.0)
        V.memset(zmb13[:, :, :, 0:1], 0.0)
        V.memset(pmb2[:], 1.0)
        for k2 in range(l):
            V.memset(pmb2[:, :, :, off2[k2]:off2[k2] + 1], 0.0)
        V.memset(wsu[:], 0.0)
        V.memset(wsp[:], 0.0)

        # ---------------- iterations ----------------
        for it in range(repeats):
            lo, hi = it + 1, SLAB - 1 - it
            if NCORES == 1:
                lo, hi = G, G + ROWS
            ablo = max(lo - 1, 0)

            if it == 0:
                nc.sync.dma_start(wsu[0:P - 1, ablo:hi].unsqueeze(2),
                                  ubar[1:P, ablo:hi, 0:1])

            # ======== A phase: parabola ========
            for (alo, ahi) in _blocks(ablo, hi, AB):
                R = ahi - alo

                u1 = atile("u1")
                u2 = atile("u2")
                u3 = atile("u3")
                tm = atile("tm")
                # u3 = p3 + sigmap*dz(ubar)
                V.tensor_tensor(u3[:, :R, :, 0:L - 1],
                                ubar[:, alo:ahi, :, 1:L],
                                ubar[:, alo:ahi, :, 0:L - 1], op=OP.subtract)
                V.memset(u3[:, :R, :, L - 1:L], 0.0)
                V.tensor_scalar_mul(u3[:, :R], u3[:, :R], sigmap)
                if it > 0:
                    V.tensor_tensor(u3[:, :R], u3[:, :R], p3[:, alo:ahi],
                                    op=OP.add)

                # u1 = p1 + sigmap*(dh(ubar)*mA) + (sigmap/tau)*msum1~
                V.tensor_tensor(u1[:, :R], ubar[:, alo + 1:ahi + 1],
                                ubar[:, alo:ahi], op=OP.subtract)
                if not (alo >= G and ahi <= SLAB - G - 1):
                    V.tensor_tensor(u1[:, :R], u1[:, :R], mAx[:, alo:ahi],
                                    op=OP.mult)
                if it > 0:
                    V.tensor_tensor(u1[:, :R], u1[:, :R], msum1[:, alo:ahi],
                                    op=OP.add)
                V.tensor_scalar_mul(u1[:, :R], u1[:, :R], sigmap)
                if it > 0:
                    V.tensor_tensor(u1[:, :R], u1[:, :R], p1[:, alo:ahi],
                                    op=OP.add)
                # u2 = p2 + sigmap*dw(ubar) + (sigmap/tau)*msum2~
                if C > 1:
                    V.tensor_tensor(u2[:, :R, 0:C - 1],
                                    ubar[:, alo:ahi, 1:C],
                                    ubar[:, alo:ahi, 0:C - 1], op=OP.subtract)
                V.scalar_tensor_tensor(u2[:, :R, C - 1:C],
                                       ubar[:, alo:ahi, C - 1:C],
                                       wm[:, 1:2], wsu[:, alo:ahi].unsqueeze(2),
                                       op0=OP.mult, op1=OP.add)
                if it > 0:
                    V.tensor_tensor(u2[:, :R], u2[:, :R], msum2[:, alo:ahi],
                                    op=OP.add)
                V.tensor_scalar_mul(u2[:, :R], u2[:, :R], sigmap)
                if it > 0:
                    V.tensor_tensor(u2[:, :R], u2[:, :R], p2[:, alo:ahi],
                                    op=OP.add)
                # --- cubic setup (square/relu: any table set) ---
                q2 = atile("q2")
                S.activation(q2[:, :R], u1[:, :R], AF.Square)
                S.activation(tm[:, :R], u2[:, :R], AF.Square)
                V.tensor_tensor(q2[:, :R], q2[:, :R], tm[:, :R], op=OP.add)
                bv = atile("tm")
                V.tensor_scalar_mul(bv[:, :R], q2[:, :R], 0.25)
                V.tensor_tensor(bv[:, :R], bv[:, :R], ld2[:, alo:ahi],
                                op=OP.subtract)
                msk = atile("msk", U16)
                V.tensor_tensor(msk[:, :R], u3[:, :R], bv[:, :R], op=OP.is_lt)
                bq = atile("bq")
                V.tensor_tensor(bq[:, :R], u3[:, :R], ld2[:, alo:ahi], op=OP.add)
                V.tensor_scalar(bq[:, :R], bq[:, :R], -1.0 / 3.0, 2.0 / 3.0,
                                op0=OP.mult, op1=OP.add)
                b3 = atile("b3")
                S.activation(b3[:, :R], bq[:, :R], AF.Square)
                V.tensor_tensor(b3[:, :R], b3[:, :R], bq[:, :R], op=OP.mult)
                dd = atile("dd")
                V.tensor_scalar_mul(dd[:, :R], q2[:, :R], 0.25)
                V.tensor_tensor(dd[:, :R], dd[:, :R], b3[:, :R], op=OP.add)
                dneg = atile("dneg", U16)
                V.tensor_scalar(dneg[:, :R], dd[:, :R], 0.0, None, op0=OP.is_lt)

                # --- ln/exp batch (no Sqrt anywhere: one act table set) ---
                lq = atile("lq")
                S.activation(lq[:, :R], q2[:, :R], AF.Ln)
                norm = atile("norm")  # = 0.5*sqrt(q2)
                S.activation(norm[:, :R], lq[:, :R], AF.Exp, scale=0.5,
                             bias=ln_half)
                rq = atile("rq")
                S.activation(rq[:, :R], lq[:, :R], AF.Exp, scale=-0.5,
                             bias=ln_two)
                sqd = atile("sqd")
                V.tensor_scalar(sqd[:, :R], dd[:, :R], 0.0, None, op0=OP.max)
                S.activation(sqd[:, :R], sqd[:, :R], AF.Ln)
                S.activation(sqd[:, :R], sqd[:, :R], AF.Exp, scale=0.5)
                lnb = atile("lnb")
                S.activation(lnb[:, :R], bq[:, :R], AF.Ln, scale=-1.0)
                sb2 = atile("b3")  # 2*sqrt(-bq) = exp(0.5*lnb + ln2)
                S.activation(sb2[:, :R], lnb[:, :R], AF.Exp, scale=0.5,
                             bias=ln_two)
                aa = atile("dd")  # reuse dd
                V.tensor_tensor(aa[:, :R], norm[:, :R], sqd[:, :R], op=OP.add)
                lt = atile("sqd")  # reuse sqd
                S.activation(lt[:, :R], aa[:, :R], AF.Ln)
                cc = atile("cc")
                S.activation(cc[:, :R], lt[:, :R], AF.Exp, scale=1.0 / 3.0)
                rc = atile("rc")
                S.activation(rc[:, :R], lt[:, :R], AF.Exp, scale=-1.0 / 3.0)
                vv = atile("vv")
                V.tensor_tensor(vv[:, :R], bq[:, :R], rc[:, :R], op=OP.mult)
                V.tensor_tensor(vv[:, :R], cc[:, :R], vv[:, :R], op=OP.subtract)
                # ratio = clip(0.5*norm*(-bq)^{-1.5}, <=1)
                eb = atile("rc")  # reuse rc
                S.activation(eb[:, :R], lnb[:, :R], AF.Exp, scale=-1.5)
                rat = atile("rat")
                V.tensor_tensor(rat[:, :R], norm[:, :R], eb[:, :R], op=OP.mult)
                V.tensor_scalar(rat[:, :R], rat[:, :R], 1.0, None, op0=OP.min)
                # y = t^2 = exp(ln(1-r) - ln(1+r))
                l1m = atile("cc")
                S.activation(l1m[:, :R], rat[:, :R], AF.Ln, scale=-1.0, bias=1.0)
                l1p = atile("dd")
                S.activation(l1p[:, :R], rat[:, :R], AF.Ln, scale=1.0, bias=1.0)
                V.tensor_tensor(l1m[:, :R], l1m[:, :R], l1p[:, :R],
                                op=OP.subtract)
                S.activation(rat[:, :R], l1m[:, :R], AF.Exp)

                # --- cos((2/3)atan(sqrt(y))) as deg-4 poly in y (in rat) ---
                PC = (0.99981162, -0.21556342, 0.11681845, -0.03518031)
                cs3 = atile("cc")
                V.tensor_scalar(cs3[:, :R], rat[:, :R], PC[3], PC[2],
                                op0=OP.mult, op1=OP.add)
                for cof in (PC[1],):
                    V.tensor_tensor(cs3[:, :R], cs3[:, :R], rat[:, :R],
                                    op=OP.mult)
                    V.tensor_scalar(cs3[:, :R], cs3[:, :R], cof, None,
                                    op0=OP.add)
                V.tensor_tensor(rat[:, :R], cs3[:, :R], rat[:, :R],
                                op=OP.mult)
                V.tensor_scalar(rat[:, :R], rat[:, :R], PC[0], None,
                                op0=OP.add)

                # --- finish (DVE + square/copy only) ---
                V.tensor_tensor(sb2[:, :R], sb2[:, :R], rat[:, :R], op=OP.mult)
                V.copy_predicated(vv[:, :R], dneg[:, :R], sb2[:, :R])
                # scl = vv * 2/norm
                V.tensor_tensor(vv[:, :R], vv[:, :R], rq[:, :R], op=OP.mult)
                nzm = atile("dneg", U16)
                V.tensor_scalar(nzm[:, :R], q2[:, :R], 0.0, None, op0=OP.is_gt)
                V.tensor_tensor(nzm[:, :R], nzm[:, :R], msk[:, :R],
                                op=OP.logical_and)
                gu = atile("rat")
                V.tensor_tensor(gu[:, :R], vv[:, :R], u1[:, :R], op=OP.mult)
                S.activation(p1[:, alo:ahi], u1[:, :R], AF.Copy)
                V.copy_predicated(p1[:, alo:ahi], nzm[:, :R], gu[:, :R])
                V.tensor_tensor(gu[:, :R], vv[:, :R], u2[:, :R], op=OP.mult)
                S.activation(p2[:, alo:ahi], u2[:, :R], AF.Copy)
                V.copy_predicated(p2[:, alo:ahi], nzm[:, :R], gu[:, :R])
                # p3 = where(msk, 0.25*(p1n^2+p2n^2) - ld2, u3)
                S.activation(q2[:, :R], p1[:, alo:ahi], AF.Square)
                S.activation(tm[:, :R], p2[:, alo:ahi], AF.Square)
                V.tensor_tensor(q2[:, :R], q2[:, :R], tm[:, :R], op=OP.add)
                V.tensor_scalar_mul(q2[:, :R], q2[:, :R], 0.25)
                V.tensor_tensor(q2[:, :R], q2[:, :R], ld2[:, alo:ahi],
                                op=OP.subtract)
                S.activation(p3[:, alo:ahi], u3[:, :R], AF.Copy)
                V.copy_predicated(p3[:, alo:ahi], msk[:, :R], q2[:, :R])

            nc.sync.dma_start(wsp[1:P, lo:hi].unsqueeze(2),
                              p2[0:P - 1, lo:hi, C - 1:C])
            # ======== B phase: interval sums, mu update, l2proj, mu->z ====
            # (outputs only feed the next iteration's A phase: skip at the end)
            bhi_all = hi - 1 if NCORES > 1 else hi
            for (blo, bhi) in ([] if it == repeats - 1
                               else _blocks(lo, bhi_all, BB)):
                R = bhi - blo
                for (pn, sx, mux, idx) in ((p2, s2, mu2, 2), (p1, s1, mu1, 1)):
                    # z-cumsum of tau*p with leading zero column (in place)
                    zct = bt_.tile([P, BB, C, 13], F16, tag="zct",
                                   name=f"zct{idx}")
                    V.memset(zct[:, :R, :, 0:1], 0.0)
                    V.tensor_scalar_mul(zct[:, :R, :, 1:13], pn[:, blo:bhi],
                                        tau_mu)
                    V.tensor_tensor_scan(
                        flat(zct[:, :R]), flat(zmb13[:, :R]),
                        flat(zct[:, :R]), 0.0, op0=OP.mult, op1=OP.add)
                    # w = dl = tau*s - t~, t~[run k2] = ics[k2] - icz[k1]:
                    # ACT broadcast-expands ics[k2] into w (no bcast penalty),
                    # DVE does packed w = icz - w (= -t~), then w += tau*s.
                    w = btile("w")
                    for k2 in range(l):
                        S.activation(
                            w[:, :R, :, off2[k2]:off2[k2 + 1]],
                            zct[:, :R, :, k2 + 1:k2 + 2]
                            .broadcast_to([P, R, C, k2 + 1]), AF.Copy)
                    for k2 in range(l):
                        V.tensor_tensor(
                            w[:, :R, :, off2[k2]:off2[k2 + 1]],
                            zct[:, :R, :, 0:k2 + 1],
                            w[:, :R, :, off2[k2]:off2[k2 + 1]],
                            op=OP.subtract)
                    mid = bhi
                    Rm = mid - blo
                    halves = [(E, hs, ws_) for (E, hs, ws_) in
                              ((V, slice(blo, mid), slice(0, Rm)),
                               (G_, slice(mid, bhi), slice(Rm, R)))
                              if hs.stop > hs.start]
                    if it == 0:
                        # s = mu = 0: dl = w; mu' = w; m = -2w
                        for (E, hs, ws_) in halves:
                            E.tensor_copy(mux[:, hs], w[:, ws_])
                            E.tensor_scalar_mul(sx[:, hs], w[:, ws_], -2.0)
                    else:
                        ts = bt_.tile([P, BB, C, PROJ], F16, tag="csp",
                                      name=f"ts{idx}")
                        for (E, hs, ws_) in halves:
                            E.tensor_scalar_mul(ts[:, ws_], sx[:, hs], tau_mu)
                            E.tensor_tensor(w[:, ws_], w[:, ws_], ts[:, ws_],
                                            op=OP.add)
                            E.tensor_tensor(mux[:, hs], mux[:, hs],
                                            w[:, ws_], op=OP.add)
                            E.tensor_tensor(sx[:, hs], sx[:, hs],
                                            mux[:, hs], op=OP.subtract)
                            E.tensor_tensor(sx[:, hs], sx[:, hs],
                                            w[:, ws_], op=OP.subtract)
                # l2proj: s = m * nu/max(|m|, nu)   (m lives in s)
                nn = btile("w")
                tb = bt_.tile([P, BB, C, PROJ], F16, tag="csp", name="tb")
                S.activation(nn[:, :R], s1[:, blo:bhi], AF.Square)
                S.activation(tb[:, :R], s2[:, blo:bhi], AF.Square)
                V.tensor_tensor(nn[:, :R], nn[:, :R], tb[:, :R], op=OP.add)
                V.tensor_scalar(nn[:, :R], nn[:, :R], nu * nu, None,
                                op0=OP.max)
                S.activation(nn[:, :R], nn[:, :R], AF.Ln)
                S.activation(nn[:, :R], nn[:, :R], AF.Exp, scale=-0.5,
                             bias=ln_nu)
                V.tensor_tensor(s1[:, blo:bhi], s1[:, blo:bhi], nn[:, :R],
                                op=OP.mult)
                V.tensor_tensor(s2[:, blo:bhi], s2[:, blo:bhi], nn[:, :R],
                                op=OP.mult)
                # mu->z sums for next iteration's A phase
                for (mux, msum, idx) in ((mu2, msum2, 2), (mu1, msum1, 1)):
                    csp = bt_.tile([P, BB, C, PROJ], F16, tag="csp",
                                   name=f"csp{idx}")
                    V.tensor_tensor_scan(
                        flat(csp[:, :R]), flat(pmb2[:, :R]),
                        flat(mux[:, blo:bhi]), 0.0, op0=OP.mult, op1=OP.add)
                    ms = msum[:, blo:bhi]
                    for k2 in range(l - 1, -1, -1):
                        seg = csp[:, :R, :, off2[k2]:off2[k2 + 1]]
                        if k2 == l - 1:
                            V.tensor_copy(ms[:, :, :, 0:l], seg)
                        else:
                            V.tensor_tensor(ms[:, :, :, 0:k2 + 1],
                                            ms[:, :, :, 0:k2 + 1], seg,
                                            op=OP.add)

            # ======== C phase: clipping ========
            for (alo, ahi) in _blocks(lo, hi, AB):
                R = ahi - alo
                pa = atile("u1")
                pc = atile("u2")
                acc = atile("u3")
                dw = atile("q2")
                # from iter 2 on un consumes only z in [1, L-1): build the
                # divergence z-sliced (zq) as well
                zq = slice(0, L) if it <= 1 else slice(1, L - 1)
                if not (alo >= G and ahi <= SLAB - G - 1):
                    V.tensor_tensor(pa[:, :R, :, zq],
                                    p1[:, alo:ahi, :, zq],
                                    mAx[:, alo:ahi, :, zq], op=OP.mult)
                    pav = pa[:, :R, :, zq]
                else:
                    pav = p1[:, alo:ahi, :, zq]
                if not (alo - 1 >= G and ahi - 1 <= SLAB - G - 1):
                    V.tensor_tensor(pc[:, :R, :, zq],
                                    p1[:, alo - 1:ahi - 1, :, zq],
                                    mCx[:, alo - 1:ahi - 1, :, zq],
                                    op=OP.mult)
                    pcv = pc[:, :R, :, zq]
                else:
                    pcv = p1[:, alo - 1:ahi - 1, :, zq]
                V.tensor_tensor(acc[:, :R, :, zq], pav, pcv, op=OP.subtract)
                if C > 1:
                    if C > 2:
                        V.tensor_tensor(dw[:, :R, 1:C - 1, zq],
                                        p2[:, alo:ahi, 1:C - 1, zq],
                                        p2[:, alo:ahi, 0:C - 2, zq],
                                        op=OP.subtract)
                    V.scalar_tensor_tensor(dw[:, :R, C - 1:C, zq],
                                           p2[:, alo:ahi, C - 1:C, zq],
                                           wm[:, 0:1],
                                           p2[:, alo:ahi, C - 2:C - 1, zq],
                                           op0=OP.mult, op1=OP.subtract)
                    V.tensor_tensor(dw[:, :R, 0:1, zq],
                                    p2[:, alo:ahi, 0:1, zq],
                                    wsp[:, alo:ahi].unsqueeze(2)[:, :, :, zq],
                                    op=OP.subtract)
                else:
                    V.scalar_tensor_tensor(dw[:, :R, 0:1, zq],
                                           p2[:, alo:ahi, 0:1, zq],
                                           wm[:, 0:1],
                                           wsp[:, alo:ahi].unsqueeze(2)
                                           [:, :, :, zq],
                                           op0=OP.mult, op1=OP.subtract)
                V.tensor_tensor(acc[:, :R, :, zq], acc[:, :R, :, zq],
                                dw[:, :R, :, zq], op=OP.add)
                if it <= 1:
                    V.tensor_tensor(acc[:, :R, :, 0:L - 1],
                                    acc[:, :R, :, 0:L - 1],
                                    p3[:, alo:ahi, :, 0:L - 1], op=OP.add)
                    V.tensor_tensor(acc[:, :R, :, 1:L], acc[:, :R, :, 1:L],
                                    p3[:, alo:ahi, :, 0:L - 1],
                                    op=OP.subtract)
                else:
                    V.tensor_tensor(acc[:, :R, :, 1:L - 1],
                                    acc[:, :R, :, 1:L - 1],
                                    p3[:, alo:ahi, :, 1:L - 1], op=OP.add)
                    V.tensor_tensor(acc[:, :R, :, 1:L - 1],
                                    acc[:, :R, :, 1:L - 1],
                                    p3[:, alo:ahi, :, 0:L - 2],
                                    op=OP.subtract)
                # un = clip(u + tauu*acc); boundary z; ubar = 2un - u
                # from iter 2 on, z-cols 0 and L-1 of u AND ubar are
                # constant (1.0 / 0.0): update only z in [1, L-1)
                zs = slice(0, L) if it <= 1 else slice(1, L - 1)
                zw = L if it <= 1 else L - 2
                un = atile("tm")
                V.tensor_scalar_mul(un[:, :R, :, 0:zw], acc[:, :R, :, zs],
                                    tauu)
                V.tensor_tensor(un[:, :R, :, 0:zw], un[:, :R, :, 0:zw],
                                u[:, alo:ahi, :, zs], op=OP.add)
                V.tensor_scalar(un[:, :R, :, 0:zw], un[:, :R, :, 0:zw],
                                0.0, 1.0, op0=OP.max, op1=OP.min)
                if it <= 1:
                    V.memset(un[:, :R, :, 0:1], 1.0)
                    V.memset(un[:, :R, :, L - 1:L], 0.0)
                if it < repeats - 1:
                    V.tensor_scalar_mul(acc[:, :R, :, 0:zw],
                                        un[:, :R, :, 0:zw], 2.0)
                    V.tensor_tensor(ubar[:, alo:ahi, :, zs],
                                    acc[:, :R, :, 0:zw],
                                    u[:, alo:ahi, :, zs], op=OP.subtract)
                S.activation(u[:, alo:ahi, :, zs], un[:, :R, :, 0:zw],
                             AF.Copy)
                if it == repeats - 1:
                    olo, ohi = max(alo, G), min(ahi, G + ROWS)
                    if olo < ohi:
                        nc.sync.dma_start(
                            u_out.ap()[:, (olo - G) * C * L:(ohi - G) * C * L],
                            flat(u[:, olo:ohi]))

            if it < repeats - 1:
                nlo, nhi = it + 2, SLAB - 2 - it
                nablo = max(nlo - 1, 0)
                nc.sync.dma_start(wsu[0:P - 1, nablo:nhi].unsqueeze(2),
                                  ubar[1:P, nablo:nhi, 0:1])

        # (output is streamed per C-block of the last iteration)

    nc.compile()
    return nc


_cache = {}


def _get_program(lmbda, nu, repeats, l, cfg_key=None):
    key = (float(lmbda), float(nu), int(repeats), int(l))
    if key not in _cache:
        _cache[key] = build_program(float(lmbda), float(nu), int(repeats),
                                    int(l))
    return _cache[key]


def make_inputs(f, repeats, cfg=None):
    cfg = cfg or CFG
    H, W, L, NCORES, P = cfg["H"], cfg["W"], cfg["L"], cfg["NCORES"], cfg["P"]
    C = W // P
    ROWS = H // NCORES
    G = int(repeats)
    SLAB = ROWS + 2 * G
    f2 = np.asarray(f, dtype=np.float32).reshape(H, W)
    fpad = np.zeros((H + 2 * G, W), np.float32)
    fpad[G:G + H] = f2
    in_maps = []
    for k in range(NCORES):
        slab = fpad[k * ROWS: k * ROWS + SLAB]              # [SLAB, W]
        arr = slab.reshape(SLAB, P, C).transpose(1, 0, 2)   # [P, SLAB, C]
        g = np.arange(SLAB) + k * ROWS - G                  # global row ids
        mAv = ((g >= 0) & (g <= H - 2)).astype(np.float16)
        mCv = ((g >= 0) & (g <= H - 1)).astype(np.float16)
        wmv = np.ones((P, 2), np.float32)
        wmv[:, 1] = -1.0
        wmv[P - 1, :] = 0.0
        in_maps.append({
            "f_in": np.ascontiguousarray(arr.reshape(P, SLAB * C)),
            "mA_in": np.ascontiguousarray(np.broadcast_to(mAv, (P, SLAB))),
            "mC_in": np.ascontiguousarray(np.broadcast_to(mCv, (P, SLAB))),
            "wm_in": wmv,
        })
    return in_maps


def assemble_output(results, repeats, cfg=None):
    cfg = cfg or CFG
    H, W, L, NCORES, P = cfg["H"], cfg["W"], cfg["L"], cfg["NCORES"], cfg["P"]
    C = W // P
    ROWS = H // NCORES
    out = np.empty((H, W, 1, L), np.float32)
    for k in range(NCORES):
        o = results[k]["u_out"].astype(np.float32).reshape(P, ROWS, C, L)
        out[k * ROWS:(k + 1) * ROWS, :, 0, :] = (
            o.transpose(1, 0, 2, 3).reshape(ROWS, W, L))
    return out


def kernel(f, lmbda, nu, repeats, l):
    l = int(l)
    repeats = int(repeats)
    cfg = dict(CFG)
    cfg["L"] = l
    key = (float(lmbda), float(nu), repeats, l)
    if key not in _cache:
        _cache[key] = build_program(float(lmbda), float(nu), repeats, l,
                                    cfg=cfg)
    nc = _cache[key]
    in_maps = make_inputs(np.asarray(f, np.float32), repeats, cfg=cfg)
    res = run_bass_kernel_spmd(nc, in_maps,
                               core_ids=list(range(cfg["NCORES"])))
    return assemble_output(res.results, repeats, cfg=cfg)


# revision 56
# speedup vs baseline: 1.0334x; 1.0334x over previous
"""Trainium2 Bass kernel for nn_PrimalDual (primal-dual multi-label segmentation).

Strategy (v2):
  - Shard image rows (h) across 8 cores; ROWS=48 owned + G=repeats ghost rows
    each side computed redundantly (ghost shrinks 1 row/iter; no comms).
  - Layout: partition q in [0,128) holds image columns w = C*q + c; free dims
    (h_local, c, z|proj). All state SBUF-resident, f16 (u too).
  - Dual state is stored tau-scaled (s~ = tau*s, mu~ = tau*mu) and the PROJ
    axis is enumerated k2-major, so interval sums and the mu->z sums are
    forward segmented scans plus contiguous slice ops:
        t~ = intervalsum(tau*p)          (z-cumsum + 12 slice ops)
        mu~' = mu~ + (s~ - t~);  m~ = t~ - mu~'   (identity: m = s - mu - 2dl)
        msum~ = segscan(mu~') diag-accumulated over k2 runs
  - No DVE reciprocals: divisions/powers go through ACT Ln/Exp; ACT ops are
    batched per block in table-set order (sqrt -> ln/exp -> trig) to minimize
    1283ns activation-table loads.
  - Masks are uint16 (2-byte keeps DVE 2x modes; valid for copy_predicated).
  - Pool (gpsimd) engine takes a slice of the PROJ-sized adds.
"""

import numpy as np
from contextlib import ExitStack

import concourse.bass as bass
import concourse.tile as tile
from concourse import bacc, mybir
from concourse.bass_utils import run_bass_kernel_spmd

# Force the act-table pass to pick the combined ln+exp set: strip Ln/Exp from
# every other set (order/ids preserved, so walrus still sees valid indices).
_orig_get_tables = bacc.get_activation_tables


def _patched_get_tables(arch):
    tabs = {k: set(v) for k, v in _orig_get_tables(arch).items()}
    comb = "natural_log_exp_and_others"
    if comb in tabs:
        ln = mybir.ActivationFunctionType.Ln
        ex = mybir.ActivationFunctionType.Exp
        for name, fns in tabs.items():
            if name != comb:
                fns.discard(ln)
                fns.discard(ex)
    return tabs


bacc.get_activation_tables = _patched_get_tables

F16 = mybir.dt.float16
U16 = mybir.dt.uint16
F32 = mybir.dt.float32
AF = mybir.ActivationFunctionType
OP = mybir.AluOpType

CFG = dict(H=384, W=384, L=12, NCORES=8, P=128)

AB = 10   # A/C-phase row-block
BB = 15   # B-phase row-block

_HALF_PI = 1.5707963267948966


def flat(ap):
    nd = len(ap.shape)
    if nd == 2:
        return ap
    names = " ".join(f"d{i}" for i in range(nd - 1))
    return ap.rearrange(f"p {names} -> p ({names})")


def _register_consts(nc, values):
    for v in values:
        v = float(v)
        if (mybir.dt.float32, v) in nc.const_aps.aps:
            continue
        t = nc.alloc_sbuf_tensor(f"constf32-{len(nc.const_aps.aps)}", [128, 1], F32)
        nc.gpsimd.memset(t.ap(), v)
        nc.const_aps.aps[(mybir.dt.float32, v)] = t.ap()
    nc.all_engine_barrier()


def _blocks(lo, hi, step):
    out = []
    r = lo
    while r < hi:
        out.append((r, min(r + step, hi)))
        r = out[-1][1]
    return out


def build_program(lmbda, nu, repeats, l, cfg=None):
    cfg = cfg or CFG
    H, W, L, NCORES, P = cfg["H"], cfg["W"], cfg["L"], cfg["NCORES"], cfg["P"]
    assert L == l
    assert W % P == 0
    C = W // P
    ROWS = H // NCORES
    G = repeats
    SLAB = ROWS + 2 * G
    PROJ = l * (l + 1) // 2

    sigmap = 1.0 / (3.0 + l)
    tauu = 1.0 / 6.0
    tau_mu = 1.0 / (2.0 + PROJ / 4.0)
    lmbda = float(lmbda)
    nu = float(nu)
    sql = float(np.sqrt(lmbda))
    kl = [(z + 1) / l for z in range(l)]
    ln_nu = float(np.log(nu))
    ln_half = float(np.log(0.5))
    ln_two = float(np.log(2.0))

    # k2-major run offsets: run k2 holds k1 = 0..k2, length k2+1
    off2 = [k2 * (k2 + 1) // 2 for k2 in range(l + 1)]

    nc = bacc.Bacc("TRN2", target_bir_lowering=False, debug=False,
                   num_devices=NCORES)
    _register_consts(nc, [sql * k for k in kl] +
                     [_HALF_PI, ln_nu, ln_half, ln_two, 0.0])

    f_in = nc.dram_tensor("f_in", [P, SLAB * C], F32, kind="ExternalInput")
    mA_in = nc.dram_tensor("mA_in", [P, SLAB], F16, kind="ExternalInput")
    mC_in = nc.dram_tensor("mC_in", [P, SLAB], F16, kind="ExternalInput")
    wm_in = nc.dram_tensor("wm_in", [P, 2], F32, kind="ExternalInput")
    u_out = nc.dram_tensor("u_out", [P, ROWS * C * L], F16, kind="ExternalOutput")

    with tile.TileContext(nc) as tc, ExitStack() as ctx, \
            nc.allow_low_precision(reason="f16 state by design"):
        V = nc.vector
        S = nc.scalar
        G_ = nc.gpsimd

        st = ctx.enter_context(tc.tile_pool(name="state", bufs=1))
        u = st.tile([P, SLAB, C, L], F16)
        ubar = st.tile([P, SLAB, C, L], F16)
        p1 = st.tile([P, SLAB, C, L], F16)
        p2 = st.tile([P, SLAB, C, L], F16)
        p3 = st.tile([P, SLAB, C, L], F16)
        s1 = st.tile([P, SLAB, C, PROJ], F16)
        s2 = st.tile([P, SLAB, C, PROJ], F16)
        mu1 = st.tile([P, SLAB, C, PROJ], F16)
        mu2 = st.tile([P, SLAB, C, PROJ], F16)
        ld2 = st.tile([P, SLAB, C, L], F16)
        msum1 = st.tile([P, SLAB, C, L], F16)   # tau-scaled mu->z sums
        msum2 = st.tile([P, SLAB, C, L], F16)
        mAx = st.tile([P, SLAB, C, L], F16)     # expanded edge masks
        mCx = st.tile([P, SLAB, C, L], F16)
        fsb = st.tile([P, SLAB, C], F32)
        zmb13 = st.tile([P, BB, C, 13], F16)    # z-scan mask (0 at col 0)
        pmb2 = st.tile([P, BB, C, PROJ], F16)   # proj-scan mask, k2-major
        wm = st.tile([P, 2], F32)
        wsu = st.tile([P, SLAB, L], F16)
        wsp = st.tile([P, SLAB, L], F16)

        at_ = ctx.enter_context(tc.tile_pool(name="atemp", bufs=2))
        bt_ = ctx.enter_context(tc.tile_pool(name="btemp", bufs=1))

        def atile(tag, dt=F16):
            return at_.tile([P, AB, C, L], dt, tag=tag, name=tag)

        def btile(tag, dt=F16):
            return bt_.tile([P, BB, C, PROJ], dt, tag=tag, name=tag)

        # ---------------- init ----------------
        nc.sync.dma_start(flat(fsb[:]), f_in.ap())
        nc.sync.dma_start(wm[:], wm_in.ap())
        fb = fsb[:].unsqueeze(3).broadcast_to([P, SLAB, C, L])
        V.tensor_copy(u[:], fb)
        S.activation(ubar[:, 0:12], fsb[:, 0:12].unsqueeze(3)
                     .broadcast_to([P, 12, C, L]), AF.Copy)
        S.activation(ubar[:, 12:SLAB], fsb[:, 12:SLAB].unsqueeze(3)
                     .broadcast_to([P, SLAB - 12, C, L]), AF.Copy)
        for z in range(L):
            S.activation(ld2[:, :, :, z:z + 1], fsb[:].unsqueeze(3),
                         AF.Square, scale=-sql, bias=sql * kl[z])
        for t in (p1, p2, p3, s1, s2, mu1, mu2, msum1, msum2):
            G_.memset(t[:], 0.0)
        # expanded edge masks (reuse fsb staging buffer for the DMA'd rows)
        mArow = st.tile([P, SLAB], F16)
        mCrow = st.tile([P, SLAB], F16)
        nc.sync.dma_start(mArow[:], mA_in.ap())
        nc.sync.dma_start(mCrow[:], mC_in.ap())
        S.activation(mAx[:], mArow[:].unsqueeze(2).unsqueeze(3)
                     .broadcast_to([P, SLAB, C, L]), AF.Copy)
        S.activation(mCx[:], mCrow[:].unsqueeze(2).unsqueeze(3)
                     .broadcast_to([P, SLAB, C, L]), AF.Copy)
        V.memset(zmb13[:], 1.0)
        V.memset(zmb13[:, :, :, 0:1], 0.0)
        V.memset(pmb2[:], 1.0)
        for k2 in range(l):
            V.memset(pmb2[:, :, :, off2[k2]:off2[k2] + 1], 0.0)
        V.memset(wsu[:], 0.0)
        V.memset(wsp[:], 0.0)

        # ---------------- iterations ----------------
        for it in range(repeats):
            lo, hi = it + 1, SLAB - 1 - it
            if NCORES == 1:
                lo, hi = G, G + ROWS
            ablo = max(lo - 1, 0)

            if it == 0:
                nc.sync.dma_start(wsu[0:P - 1, ablo:hi].unsqueeze(2),
                                  ubar[1:P, ablo:hi, 0:1])

            # ======== A phase: parabola ========
            for (alo, ahi) in _blocks(ablo, hi, AB):
                R = ahi - alo

                u1 = atile("u1")
                u2 = atile("u2")
                u3 = atile("u3")
                tm = atile("tm")
                # u3 = p3 + sigmap*dz(ubar)
                V.tensor_tensor(u3[:, :R, :, 0:L - 1],
                                ubar[:, alo:ahi, :, 1:L],
                                ubar[:, alo:ahi, :, 0:L - 1], op=OP.subtract)
                V.memset(u3[:, :R, :, L - 1:L], 0.0)
                V.tensor_scalar_mul(u3[:, :R], u3[:, :R], sigmap)
                if it > 0:
                    V.tensor_tensor(u3[:, :R], u3[:, :R], p3[:, alo:ahi],
                                    op=OP.add)

                # u1 = p1 + sigmap*(dh(ubar)*mA) + (sigmap/tau)*msum1~
                V.tensor_tensor(u1[:, :R], ubar[:, alo + 1:ahi + 1],
                                ubar[:, alo:ahi], op=OP.subtract)
                if not (alo >= G and ahi <= SLAB - G - 1):
                    V.tensor_tensor(u1[:, :R], u1[:, :R], mAx[:, alo:ahi],
                                    op=OP.mult)
                if it > 0:
                    V.tensor_tensor(u1[:, :R], u1[:, :R], msum1[:, alo:ahi],
                                    op=OP.add)
                V.tensor_scalar_mul(u1[:, :R], u1[:, :R], sigmap)
                if it > 0:
                    V.tensor_tensor(u1[:, :R], u1[:, :R], p1[:, alo:ahi],
                                    op=OP.add)
                # u2 = p2 + sigmap*dw(ubar) + (sigmap/tau)*msum2~
                if C > 1:
                    V.tensor_tensor(u2[:, :R, 0:C - 1],
                                    ubar[:, alo:ahi, 1:C],
                                    ubar[:, alo:ahi, 0:C - 1], op=OP.subtract)
                V.scalar_tensor_tensor(u2[:, :R, C - 1:C],
                                       ubar[:, alo:ahi, C - 1:C],
                                       wm[:, 1:2], wsu[:, alo:ahi].unsqueeze(2),
                                       op0=OP.mult, op1=OP.add)
                if it > 0:
                    V.tensor_tensor(u2[:, :R], u2[:, :R], msum2[:, alo:ahi],
                                    op=OP.add)
                V.tensor_scalar_mul(u2[:, :R], u2[:, :R], sigmap)
                if it > 0:
                    V.tensor_tensor(u2[:, :R], u2[:, :R], p2[:, alo:ahi],
                                    op=OP.add)
                # --- cubic setup (square/relu: any table set) ---
                q2 = atile("q2")
                S.activation(q2[:, :R], u1[:, :R], AF.Square)
                S.activation(tm[:, :R], u2[:, :R], AF.Square)
                V.tensor_tensor(q2[:, :R], q2[:, :R], tm[:, :R], op=OP.add)
                bv = atile("tm")
                V.tensor_scalar_mul(bv[:, :R], q2[:, :R], 0.25)
                V.tensor_tensor(bv[:, :R], bv[:, :R], ld2[:, alo:ahi],
                                op=OP.subtract)
                msk = atile("msk", U16)
                V.tensor_tensor(msk[:, :R], u3[:, :R], bv[:, :R], op=OP.is_lt)
                bq = atile("bq")
                V.tensor_tensor(bq[:, :R], u3[:, :R], ld2[:, alo:ahi], op=OP.add)
                V.tensor_scalar(bq[:, :R], bq[:, :R], -1.0 / 3.0, 2.0 / 3.0,
                                op0=OP.mult, op1=OP.add)
                b3 = atile("b3")
                S.activation(b3[:, :R], bq[:, :R], AF.Square)
                V.tensor_tensor(b3[:, :R], b3[:, :R], bq[:, :R], op=OP.mult)
                dd = atile("dd")
                V.tensor_scalar_mul(dd[:, :R], q2[:, :R], 0.25)
                V.tensor_tensor(dd[:, :R], dd[:, :R], b3[:, :R], op=OP.add)
                dneg = atile("dneg", U16)
                V.tensor_scalar(dneg[:, :R], dd[:, :R], 0.0, None, op0=OP.is_lt)

                # --- ln/exp batch (no Sqrt anywhere: one act table set) ---
                lq = atile("lq")
                S.activation(lq[:, :R], q2[:, :R], AF.Ln)
                norm = atile("norm")  # = 0.5*sqrt(q2)
                S.activation(norm[:, :R], lq[:, :R], AF.Exp, scale=0.5,
                             bias=ln_half)
                rq = atile("rq")
                S.activation(rq[:, :R], lq[:, :R], AF.Exp, scale=-0.5,
                             bias=ln_two)
                sqd = atile("sqd")
                V.tensor_scalar(sqd[:, :R], dd[:, :R], 0.0, None, op0=OP.max)
                S.activation(sqd[:, :R], sqd[:, :R], AF.Ln)
                S.activation(sqd[:, :R], sqd[:, :R], AF.Exp, scale=0.5)
                lnb = atile("lnb")
                S.activation(lnb[:, :R], bq[:, :R], AF.Ln, scale=-1.0)
                sb2 = atile("b3")  # 2*sqrt(-bq) = exp(0.5*lnb + ln2)
                S.activation(sb2[:, :R], lnb[:, :R], AF.Exp, scale=0.5,
                             bias=ln_two)
                aa = atile("dd")  # reuse dd
                V.tensor_tensor(aa[:, :R], norm[:, :R], sqd[:, :R], op=OP.add)
                lt = atile("sqd")  # reuse sqd
                S.activation(lt[:, :R], aa[:, :R], AF.Ln)
                cc = atile("cc")
                S.activation(cc[:, :R], lt[:, :R], AF.Exp, scale=1.0 / 3.0)
                rc = atile("rc")
                S.activation(rc[:, :R], lt[:, :R], AF.Exp, scale=-1.0 / 3.0)
                vv = atile("vv")
                V.tensor_tensor(vv[:, :R], bq[:, :R], rc[:, :R], op=OP.mult)
                V.tensor_tensor(vv[:, :R], cc[:, :R], vv[:, :R], op=OP.subtract)
                # ratio = clip(0.5*norm*(-bq)^{-1.5}, <=1)
                eb = atile("rc")  # reuse rc
                S.activation(eb[:, :R], lnb[:, :R], AF.Exp, scale=-1.5)
                rat = atile("rat")
                V.tensor_tensor(rat[:, :R], norm[:, :R], eb[:, :R], op=OP.mult)
                V.tensor_scalar(rat[:, :R], rat[:, :R], 1.0, None, op0=OP.min)
                # y = t^2 = exp(ln(1-r) - ln(1+r))
                l1m = atile("cc")
                S.activation(l1m[:, :R], rat[:, :R], AF.Ln, scale=-1.0, bias=1.0)
                l1p = atile("dd")
                S.activation(l1p[:, :R], rat[:, :R], AF.Ln, scale=1.0, bias=1.0)
                V.tensor_tensor(l1m[:, :R], l1m[:, :R], l1p[:, :R],
                                op=OP.subtract)
                S.activation(rat[:, :R], l1m[:, :R], AF.Exp)

                # --- cos((2/3)atan(sqrt(y))) as deg-4 poly in y (in rat) ---
                PC = (0.99981162, -0.21556342, 0.11681845, -0.03518031)
                cs3 = atile("cc")
                V.tensor_scalar(cs3[:, :R], rat[:, :R], PC[3], PC[2],
                                op0=OP.mult, op1=OP.add)
                for cof in (PC[1],):
                    V.tensor_tensor(cs3[:, :R], cs3[:, :R], rat[:, :R],
                                    op=OP.mult)
                    V.tensor_scalar(cs3[:, :R], cs3[:, :R], cof, None,
                                    op0=OP.add)
                V.tensor_tensor(rat[:, :R], cs3[:, :R], rat[:, :R],
                                op=OP.mult)
                V.tensor_scalar(rat[:, :R], rat[:, :R], PC[0], None,
                                op0=OP.add)

                # --- finish (DVE + square/copy only) ---
                V.tensor_tensor(sb2[:, :R], sb2[:, :R], rat[:, :R], op=OP.mult)
                V.copy_predicated(vv[:, :R], dneg[:, :R], sb2[:, :R])
                # scl = vv * 2/norm
                V.tensor_tensor(vv[:, :R], vv[:, :R], rq[:, :R], op=OP.mult)
                nzm = atile("dneg", U16)
                V.tensor_scalar(nzm[:, :R], q2[:, :R], 0.0, None, op0=OP.is_gt)
                V.tensor_tensor(nzm[:, :R], nzm[:, :R], msk[:, :R],
                                op=OP.logical_and)
                gu = atile("rat")
                V.tensor_tensor(gu[:, :R], vv[:, :R], u1[:, :R], op=OP.mult)
                S.activation(p1[:, alo:ahi], u1[:, :R], AF.Copy)
                V.copy_predicated(p1[:, alo:ahi], nzm[:, :R], gu[:, :R])
                V.tensor_tensor(gu[:, :R], vv[:, :R], u2[:, :R], op=OP.mult)
                S.activation(p2[:, alo:ahi], u2[:, :R], AF.Copy)
                V.copy_predicated(p2[:, alo:ahi], nzm[:, :R], gu[:, :R])
                # p3 = where(msk, 0.25*(p1n^2+p2n^2) - ld2, u3)
                S.activation(q2[:, :R], p1[:, alo:ahi], AF.Square)
                S.activation(tm[:, :R], p2[:, alo:ahi], AF.Square)
                V.tensor_tensor(q2[:, :R], q2[:, :R], tm[:, :R], op=OP.add)
                V.tensor_scalar_mul(q2[:, :R], q2[:, :R], 0.25)
                V.tensor_tensor(q2[:, :R], q2[:, :R], ld2[:, alo:ahi],
                                op=OP.subtract)
                S.activation(p3[:, alo:ahi], u3[:, :R], AF.Copy)
                V.copy_predicated(p3[:, alo:ahi], msk[:, :R], q2[:, :R])

            nc.sync.dma_start(wsp[1:P, lo:hi].unsqueeze(2),
                              p2[0:P - 1, lo:hi, C - 1:C])
            # ======== B phase: interval sums, mu update, l2proj, mu->z ====
            # (outputs only feed the next iteration's A phase: skip at the end)
            bhi_all = hi - 1 if NCORES > 1 else hi
            for (blo, bhi) in ([] if it == repeats - 1
                               else _blocks(lo, bhi_all, BB)):
                R = bhi - blo
                for (pn, sx, mux, idx) in ((p2, s2, mu2, 2), (p1, s1, mu1, 1)):
                    # z-cumsum of tau*p with leading zero column (in place)
                    zct = bt_.tile([P, BB, C, 13], F16, tag="zct",
                                   name=f"zct{idx}")
                    V.memset(zct[:, :R, :, 0:1], 0.0)
                    V.tensor_scalar_mul(zct[:, :R, :, 1:13], pn[:, blo:bhi],
                                        tau_mu)
                    V.tensor_tensor_scan(
                        flat(zct[:, :R]), flat(zmb13[:, :R]),
                        flat(zct[:, :R]), 0.0, op0=OP.mult, op1=OP.add)
                    # w = dl = tau*s - t~, t~[run k2] = ics[k2] - icz[k1]:
                    # ACT broadcast-expands ics[k2] into w (no bcast penalty),
                    # DVE does packed w = icz - w (= -t~), then w += tau*s.
                    w = btile("w")
                    for k2 in range(l):
                        S.activation(
                            w[:, :R, :, off2[k2]:off2[k2 + 1]],
                            zct[:, :R, :, k2 + 1:k2 + 2]
                            .broadcast_to([P, R, C, k2 + 1]), AF.Copy)
                    for k2 in range(l):
                        V.tensor_tensor(
                            w[:, :R, :, off2[k2]:off2[k2 + 1]],
                            zct[:, :R, :, 0:k2 + 1],
                            w[:, :R, :, off2[k2]:off2[k2 + 1]],
                            op=OP.subtract)
                    mid = bhi
                    Rm = mid - blo
                    halves = [(E, hs, ws_) for (E, hs, ws_) in
                              ((V, slice(blo, mid), slice(0, Rm)),
                               (G_, slice(mid, bhi), slice(Rm, R)))
                              if hs.stop > hs.start]
                    if it == 0:
                        # s = mu = 0: dl = w; mu' = w; m = -2w
                        for (E, hs, ws_) in halves:
                            E.tensor_copy(mux[:, hs], w[:, ws_])
                            E.tensor_scalar_mul(sx[:, hs], w[:, ws_], -2.0)
                    else:
                        ts = bt_.tile([P, BB, C, PROJ], F16, tag="csp",
                                      name=f"ts{idx}")
                        for (E, hs, ws_) in halves:
                            E.tensor_scalar_mul(ts[:, ws_], sx[:, hs], tau_mu)
                            E.tensor_tensor(w[:, ws_], w[:, ws_], ts[:, ws_],
                                            op=OP.add)
                            E.tensor_tensor(mux[:, hs], mux[:, hs],
                                            w[:, ws_], op=OP.add)
                            E.tensor_tensor(sx[:, hs], sx[:, hs],
                                            mux[:, hs], op=OP.subtract)
                            E.tensor_tensor(sx[:, hs], sx[:, hs],
                                            w[:, ws_], op=OP.subtract)
                # l2proj: s = m * nu/max(|m|, nu)   (m lives in s)
                nn = btile("w")
                tb = bt_.tile([P, BB, C, PROJ], F16, tag="csp", name="tb")
                S.activation(nn[:, :R], s1[:, blo:bhi], AF.Square)
                S.activation(tb[:, :R], s2[:, blo:bhi], AF.Square)
                V.tensor_tensor(nn[:, :R], nn[:, :R], tb[:, :R], op=OP.add)
                V.tensor_scalar(nn[:, :R], nn[:, :R], nu * nu, None,
                                op0=OP.max)
                S.activation(nn[:, :R], nn[:, :R], AF.Ln)
                S.activation(nn[:, :R], nn[:, :R], AF.Exp, scale=-0.5,
                             bias=ln_nu)
                V.tensor_tensor(s1[:, blo:bhi], s1[:, blo:bhi], nn[:, :R],
                                op=OP.mult)
                V.tensor_tensor(s2[:, blo:bhi], s2[:, blo:bhi], nn[:, :R],
                                op=OP.mult)
                # mu->z sums for next iteration's A phase
                for (mux, msum, idx) in ((mu2, msum2, 2), (mu1, msum1, 1)):
                    csp = bt_.tile([P, BB, C, PROJ], F16, tag="csp",
                                   name=f"csp{idx}")
                    V.tensor_tensor_scan(
                        flat(csp[:, :R]), flat(pmb2[:, :R]),
                        flat(mux[:, blo:bhi]), 0.0, op0=OP.mult, op1=OP.add)
                    ms = msum[:, blo:bhi]
                    for k2 in range(l - 1, -1, -1):
                        seg = csp[:, :R, :, off2[k2]:off2[k2 + 1]]
                        if k2 == l - 1:
                            V.tensor_copy(ms[:, :, :, 0:l], seg)
                        else:
                            V.tensor_tensor(ms[:, :, :, 0:k2 + 1],
                                            ms[:, :, :, 0:k2 + 1], seg,
                                            op=OP.add)

            # ======== C phase: clipping ========
            for (alo, ahi) in _blocks(lo, hi, AB):
                R = ahi - alo
                pa = atile("u1")
                pc = atile("u2")
                acc = atile("u3")
                dw = atile("q2")
                # from iter 2 on un consumes only z in [1, L-1): build the
                # divergence z-sliced (zq) as well
                zq = slice(0, L) if it <= 1 else slice(1, L - 1)
                if not (alo >= G and ahi <= SLAB - G - 1):
                    V.tensor_tensor(pa[:, :R, :, zq],
                                    p1[:, alo:ahi, :, zq],
                                    mAx[:, alo:ahi, :, zq], op=OP.mult)
                    pav = pa[:, :R, :, zq]
                else:
                    pav = p1[:, alo:ahi, :, zq]
                if not (alo - 1 >= G and ahi - 1 <= SLAB - G - 1):
                    V.tensor_tensor(pc[:, :R, :, zq],
                                    p1[:, alo - 1:ahi - 1, :, zq],
                                    mCx[:, alo - 1:ahi - 1, :, zq],
                                    op=OP.mult)
                    pcv = pc[:, :R, :, zq]
                else:
                    pcv = p1[:, alo - 1:ahi - 1, :, zq]
                V.tensor_tensor(acc[:, :R, :, zq], pav, pcv, op=OP.subtract)
                if C > 1:
                    if C > 2:
                        V.tensor_tensor(dw[:, :R, 1:C - 1, zq],
                                        p2[:, alo:ahi, 1:C - 1, zq],
                                        p2[:, alo:ahi, 0:C - 2, zq],
                                        op=OP.subtract)
                    V.scalar_tensor_tensor(dw[:, :R, C - 1:C, zq],
                                           p2[:, alo:ahi, C - 1:C, zq],
                                           wm[:, 0:1],
                                           p2[:, alo:ahi, C - 2:C - 1, zq],
                                           op0=OP.mult, op1=OP.subtract)
                    V.tensor_tensor(dw[:, :R, 0:1, zq],
                                    p2[:, alo:ahi, 0:1, zq],
                                    wsp[:, alo:ahi].unsqueeze(2)[:, :, :, zq],
                                    op=OP.subtract)
                else:
                    V.scalar_tensor_tensor(dw[:, :R, 0:1, zq],
                                           p2[:, alo:ahi, 0:1, zq],
                                           wm[:, 0:1],
                                           wsp[:, alo:ahi].unsqueeze(2)
                                           [:, :, :, zq],
                                           op0=OP.mult, op1=OP.subtract)
                V.tensor_tensor(acc[:, :R, :, zq], acc[:, :R, :, zq],
                                dw[:, :R, :, zq], op=OP.add)
                if it <= 1:
                    V.tensor_tensor(acc[:, :R, :, 0:L - 1],
                                    acc[:, :R, :, 0:L - 1],
                                    p3[:, alo:ahi, :, 0:L - 1], op=OP.add)
                    V.tensor_tensor(acc[:, :R, :, 1:L], acc[:, :R, :, 1:L],
                                    p3[:, alo:ahi, :, 0:L - 1],
                                    op=OP.subtract)
                else:
                    V.tensor_tensor(acc[:, :R, :, 1:L - 1],
                                    acc[:, :R, :, 1:L - 1],
                                    p3[:, alo:ahi, :, 1:L - 1], op=OP.add)
                    V.tensor_tensor(acc[:, :R, :, 1:L - 1],
                                    acc[:, :R, :, 1:L - 1],
                                    p3[:, alo:ahi, :, 0:L - 2],
                                    op=OP.subtract)
                # un = clip(u + tauu*acc); boundary z; ubar = 2un - u
                # from iter 2 on, z-cols 0 and L-1 of u AND ubar are
                # constant (1.0 / 0.0): update only z in [1, L-1)
                zs = slice(0, L) if it <= 1 else slice(1, L - 1)
                zw = L if it <= 1 else L - 2
                un = atile("tm")
                V.tensor_scalar_mul(un[:, :R, :, 0:zw], acc[:, :R, :, zs],
                                    tauu)
                V.tensor_tensor(un[:, :R, :, 0:zw], un[:, :R, :, 0:zw],
                                u[:, alo:ahi, :, zs], op=OP.add)
                V.tensor_scalar(un[:, :R, :, 0:zw], un[:, :R, :, 0:zw],
                                0.0, 1.0, op0=OP.max, op1=OP.min)
                if it <= 1:
                    V.memset(un[:, :R, :, 0:1], 1.0)
                    V.memset(un[:, :R, :, L - 1:L], 0.0)
                if it < repeats - 1:
                    V.tensor_scalar_mul(acc[:, :R, :, 0:zw],
                                        un[:, :R, :, 0:zw], 2.0)
                    V.tensor_tensor(ubar[:, alo:ahi, :, zs],
                                    acc[:, :R, :, 0:zw],
                                    u[:, alo:ahi, :, zs], op=OP.subtract)
                S.activation(u[:, alo:ahi, :, zs], un[:, :R, :, 0:zw],
                             AF.Copy)
                if it == repeats - 1:
                    olo, ohi = max(alo, G), min(ahi, G + ROWS)
                    if olo < ohi:
                        nc.sync.dma_start(
                            u_out.ap()[:, (olo - G) * C * L:(ohi - G) * C * L],
                            flat(u[:, olo:ohi]))

            if it < repeats - 1:
                nlo, nhi = it + 2, SLAB - 2 - it
                nablo = max(nlo - 1, 0)
                nc.sync.dma_start(wsu[0:P - 1, nablo:nhi].unsqueeze(2),
                                  ubar[1:P, nablo:nhi, 0:1])

        # (output is streamed per C-block of the last iteration)

    nc.compile()
    return nc


_cache = {}


def _get_program(lmbda, nu, repeats, l, cfg_key=None):
    key = (float(lmbda), float(nu), int(repeats), int(l))
    if key not in _cache:
        _cache[key] = build_program(float(lmbda), float(nu), int(repeats),
                                    int(l))
    return _cache[key]


def make_inputs(f, repeats, cfg=None):
    cfg = cfg or CFG
    H, W, L, NCORES, P = cfg["H"], cfg["W"], cfg["L"], cfg["NCORES"], cfg["P"]
    C = W // P
    ROWS = H // NCORES
    G = int(repeats)
    SLAB = ROWS + 2 * G
    f2 = np.asarray(f, dtype=np.float32).reshape(H, W)
    fpad = np.zeros((H + 2 * G, W), np.float32)
    fpad[G:G + H] = f2
    in_maps = []
    for k in range(NCORES):
        slab = fpad[k * ROWS: k * ROWS + SLAB]              # [SLAB, W]
        arr = slab.reshape(SLAB, P, C).transpose(1, 0, 2)   # [P, SLAB, C]
        g = np.arange(SLAB) + k * ROWS - G                  # global row ids
        mAv = ((g >= 0) & (g <= H - 2)).astype(np.float16)
        mCv = ((g >= 0) & (g <= H - 1)).astype(np.float16)
        wmv = np.ones((P, 2), np.float32)
        wmv[:, 1] = -1.0
        wmv[P - 1, :] = 0.0
        in_maps.append({
            "f_in": np.ascontiguousarray(arr.reshape(P, SLAB * C)),
            "mA_in": np.ascontiguousarray(np.broadcast_to(mAv, (P, SLAB))),
            "mC_in": np.ascontiguousarray(np.broadcast_to(mCv, (P, SLAB))),
            "wm_in": wmv,
        })
    return in_maps


def assemble_output(results, repeats, cfg=None):
    cfg = cfg or CFG
    H, W, L, NCORES, P = cfg["H"], cfg["W"], cfg["L"], cfg["NCORES"], cfg["P"]
    C = W // P
    ROWS = H // NCORES
    out = np.empty((H, W, 1, L), np.float32)
    for k in range(NCORES):
        o = results[k]["u_out"].astype(np.float32).reshape(P, ROWS, C, L)
        out[k * ROWS:(k + 1) * ROWS, :, 0, :] = (
            o.transpose(1, 0, 2, 3).reshape(ROWS, W, L))
    return out


def kernel(f, lmbda, nu, repeats, l):
    l = int(l)
    repeats = int(repeats)
    cfg = dict(CFG)
    cfg["L"] = l
    key = (float(lmbda), float(nu), repeats, l)
    if key not in _cache:
        _cache[key] = build_program(float(lmbda), float(nu), repeats, l,
                                    cfg=cfg)
    nc = _cache[key]
    in_maps = make_inputs(np.asarray(f, np.float32), repeats, cfg=cfg)
    res = run_bass_kernel_spmd(nc, in_maps,
                               core_ids=list(range(cfg["NCORES"])))
    return assemble_output(res.results, repeats, cfg=cfg)


# revision 57
# speedup vs baseline: 1.0339x; 1.0004x over previous
"""Trainium2 Bass kernel for nn_PrimalDual (primal-dual multi-label segmentation).

Strategy (v2):
  - Shard image rows (h) across 8 cores; ROWS=48 owned + G=repeats ghost rows
    each side computed redundantly (ghost shrinks 1 row/iter; no comms).
  - Layout: partition q in [0,128) holds image columns w = C*q + c; free dims
    (h_local, c, z|proj). All state SBUF-resident, f16 (u too).
  - Dual state is stored tau-scaled (s~ = tau*s, mu~ = tau*mu) and the PROJ
    axis is enumerated k2-major, so interval sums and the mu->z sums are
    forward segmented scans plus contiguous slice ops:
        t~ = intervalsum(tau*p)          (z-cumsum + 12 slice ops)
        mu~' = mu~ + (s~ - t~);  m~ = t~ - mu~'   (identity: m = s - mu - 2dl)
        msum~ = segscan(mu~') diag-accumulated over k2 runs
  - No DVE reciprocals: divisions/powers go through ACT Ln/Exp; ACT ops are
    batched per block in table-set order (sqrt -> ln/exp -> trig) to minimize
    1283ns activation-table loads.
  - Masks are uint16 (2-byte keeps DVE 2x modes; valid for copy_predicated).
  - Pool (gpsimd) engine takes a slice of the PROJ-sized adds.
"""

import numpy as np
from contextlib import ExitStack

import concourse.bass as bass
import concourse.tile as tile
from concourse import bacc, mybir
from concourse.bass_utils import run_bass_kernel_spmd

# Force the act-table pass to pick the combined ln+exp set: strip Ln/Exp from
# every other set (order/ids preserved, so walrus still sees valid indices).
_orig_get_tables = bacc.get_activation_tables


def _patched_get_tables(arch):
    tabs = {k: set(v) for k, v in _orig_get_tables(arch).items()}
    comb = "natural_log_exp_and_others"
    if comb in tabs:
        ln = mybir.ActivationFunctionType.Ln
        ex = mybir.ActivationFunctionType.Exp
        for name, fns in tabs.items():
            if name != comb:
                fns.discard(ln)
                fns.discard(ex)
    return tabs


bacc.get_activation_tables = _patched_get_tables

F16 = mybir.dt.float16
U16 = mybir.dt.uint16
F32 = mybir.dt.float32
AF = mybir.ActivationFunctionType
OP = mybir.AluOpType

CFG = dict(H=384, W=384, L=12, NCORES=8, P=128)

AB = 10   # A/C-phase row-block
BB = 15   # B-phase row-block

_HALF_PI = 1.5707963267948966


def flat(ap):
    nd = len(ap.shape)
    if nd == 2:
        return ap
    names = " ".join(f"d{i}" for i in range(nd - 1))
    return ap.rearrange(f"p {names} -> p ({names})")


def _register_consts(nc, values):
    for v in values:
        v = float(v)
        if (mybir.dt.float32, v) in nc.const_aps.aps:
            continue
        t = nc.alloc_sbuf_tensor(f"constf32-{len(nc.const_aps.aps)}", [128, 1], F32)
        nc.gpsimd.memset(t.ap(), v)
        nc.const_aps.aps[(mybir.dt.float32, v)] = t.ap()
    nc.all_engine_barrier()


def _blocks(lo, hi, step):
    out = []
    r = lo
    while r < hi:
        out.append((r, min(r + step, hi)))
        r = out[-1][1]
    return out


def build_program(lmbda, nu, repeats, l, cfg=None):
    cfg = cfg or CFG
    H, W, L, NCORES, P = cfg["H"], cfg["W"], cfg["L"], cfg["NCORES"], cfg["P"]
    assert L == l
    assert W % P == 0
    C = W // P
    ROWS = H // NCORES
    G = repeats
    SLAB = ROWS + 2 * G
    PROJ = l * (l + 1) // 2

    sigmap = 1.0 / (3.0 + l)
    tauu = 1.0 / 6.0
    tau_mu = 1.0 / (2.0 + PROJ / 4.0)
    lmbda = float(lmbda)
    nu = float(nu)
    sql = float(np.sqrt(lmbda))
    kl = [(z + 1) / l for z in range(l)]
    ln_nu = float(np.log(nu))
    ln_half = float(np.log(0.5))
    ln_two = float(np.log(2.0))

    # k2-major run offsets: run k2 holds k1 = 0..k2, length k2+1
    off2 = [k2 * (k2 + 1) // 2 for k2 in range(l + 1)]

    nc = bacc.Bacc("TRN2", target_bir_lowering=False, debug=False,
                   num_devices=NCORES)
    _register_consts(nc, [sql * k for k in kl] +
                     [_HALF_PI, ln_nu, ln_half, ln_two, 0.0])

    f_in = nc.dram_tensor("f_in", [P, SLAB * C], F32, kind="ExternalInput")
    mA_in = nc.dram_tensor("mA_in", [P, SLAB], F16, kind="ExternalInput")
    mC_in = nc.dram_tensor("mC_in", [P, SLAB], F16, kind="ExternalInput")
    wm_in = nc.dram_tensor("wm_in", [P, 2], F32, kind="ExternalInput")
    u_out = nc.dram_tensor("u_out", [P, ROWS * C * L], F16, kind="ExternalOutput")

    with tile.TileContext(nc) as tc, ExitStack() as ctx, \
            nc.allow_low_precision(reason="f16 state by design"):
        V = nc.vector
        S = nc.scalar
        G_ = nc.gpsimd

        st = ctx.enter_context(tc.tile_pool(name="state", bufs=1))
        u = st.tile([P, SLAB, C, L], F16)
        ubar = st.tile([P, SLAB, C, L], F16)
        p1 = st.tile([P, SLAB, C, L], F16)
        p2 = st.tile([P, SLAB, C, L], F16)
        p3 = st.tile([P, SLAB, C, L], F16)
        s1 = st.tile([P, SLAB, C, PROJ], F16)
        s2 = st.tile([P, SLAB, C, PROJ], F16)
        mu1 = st.tile([P, SLAB, C, PROJ], F16)
        mu2 = st.tile([P, SLAB, C, PROJ], F16)
        ld2 = st.tile([P, SLAB, C, L], F16)
        msum1 = st.tile([P, SLAB, C, L], F16)   # tau-scaled mu->z sums
        msum2 = st.tile([P, SLAB, C, L], F16)
        mAx = st.tile([P, SLAB, C, L], F16)     # expanded edge masks
        mCx = st.tile([P, SLAB, C, L], F16)
        fsb = st.tile([P, SLAB, C], F32)
        zmb13 = st.tile([P, BB, C, 13], F16)    # z-scan mask (0 at col 0)
        pmb2 = st.tile([P, BB, C, PROJ], F16)   # proj-scan mask, k2-major
        wm = st.tile([P, 2], F32)
        wsu = st.tile([P, SLAB, L], F16)
        wsp = st.tile([P, SLAB, L], F16)

        at_ = ctx.enter_context(tc.tile_pool(name="atemp", bufs=2))
        bt_ = ctx.enter_context(tc.tile_pool(name="btemp", bufs=1))

        def atile(tag, dt=F16):
            return at_.tile([P, AB, C, L], dt, tag=tag, name=tag)

        def btile(tag, dt=F16):
            return bt_.tile([P, BB, C, PROJ], dt, tag=tag, name=tag)

        # ---------------- init ----------------
        nc.sync.dma_start(flat(fsb[:]), f_in.ap())
        nc.sync.dma_start(wm[:], wm_in.ap())
        fb = fsb[:].unsqueeze(3).broadcast_to([P, SLAB, C, L])
        V.tensor_copy(u[:], fb)
        S.activation(ubar[:, 0:12], fsb[:, 0:12].unsqueeze(3)
                     .broadcast_to([P, 12, C, L]), AF.Copy)
        S.activation(ubar[:, 12:SLAB], fsb[:, 12:SLAB].unsqueeze(3)
                     .broadcast_to([P, SLAB - 12, C, L]), AF.Copy)
        for z in range(L):
            S.activation(ld2[:, :, :, z:z + 1], fsb[:].unsqueeze(3),
                         AF.Square, scale=-sql, bias=sql * kl[z])
        for t in (p1, p2, p3, s1, s2, mu1, mu2, msum1, msum2):
            G_.memset(t[:], 0.0)
        # expanded edge masks (reuse fsb staging buffer for the DMA'd rows)
        mArow = st.tile([P, SLAB], F16)
        mCrow = st.tile([P, SLAB], F16)
        nc.sync.dma_start(mArow[:], mA_in.ap())
        nc.sync.dma_start(mCrow[:], mC_in.ap())
        S.activation(mAx[:], mArow[:].unsqueeze(2).unsqueeze(3)
                     .broadcast_to([P, SLAB, C, L]), AF.Copy)
        S.activation(mCx[:], mCrow[:].unsqueeze(2).unsqueeze(3)
                     .broadcast_to([P, SLAB, C, L]), AF.Copy)
        V.memset(zmb13[:], 1.0)
        V.memset(zmb13[:, :, :, 0:1], 0.0)
        V.memset(pmb2[:], 1.0)
        for k2 in range(l):
            V.memset(pmb2[:, :, :, off2[k2]:off2[k2] + 1], 0.0)
        V.memset(wsu[:], 0.0)
        V.memset(wsp[:], 0.0)

        # ---------------- iterations ----------------
        for it in range(repeats):
            lo, hi = it + 1, SLAB - 1 - it
            if NCORES == 1:
                lo, hi = G, G + ROWS
            ablo = max(lo - 1, 0)

            if it == 0:
                nc.sync.dma_start(wsu[0:P - 1, ablo:hi].unsqueeze(2),
                                  ubar[1:P, ablo:hi, 0:1])

            # ======== A phase: parabola ========
            for (alo, ahi) in _blocks(ablo, hi, AB):
                R = ahi - alo

                u1 = atile("u1")
                u2 = atile("u2")
                u3 = atile("u3")
                tm = atile("tm")
                # u3 = p3 + sigmap*dz(ubar)
                if it == 0:
                    # ubar is z-constant and p3 = 0: u3 = 0 exactly
                    V.memset(u3[:, :R], 0.0)
                else:
                    V.tensor_tensor(u3[:, :R, :, 0:L - 1],
                                    ubar[:, alo:ahi, :, 1:L],
                                    ubar[:, alo:ahi, :, 0:L - 1],
                                    op=OP.subtract)
                    V.memset(u3[:, :R, :, L - 1:L], 0.0)
                    V.tensor_scalar_mul(u3[:, :R], u3[:, :R], sigmap)
                    V.tensor_tensor(u3[:, :R], u3[:, :R], p3[:, alo:ahi],
                                    op=OP.add)

                # u1 = p1 + sigmap*(dh(ubar)*mA) + (sigmap/tau)*msum1~
                V.tensor_tensor(u1[:, :R], ubar[:, alo + 1:ahi + 1],
                                ubar[:, alo:ahi], op=OP.subtract)
                if not (alo >= G and ahi <= SLAB - G - 1):
                    V.tensor_tensor(u1[:, :R], u1[:, :R], mAx[:, alo:ahi],
                                    op=OP.mult)
                if it > 0:
                    V.tensor_tensor(u1[:, :R], u1[:, :R], msum1[:, alo:ahi],
                                    op=OP.add)
                V.tensor_scalar_mul(u1[:, :R], u1[:, :R], sigmap)
                if it > 0:
                    V.tensor_tensor(u1[:, :R], u1[:, :R], p1[:, alo:ahi],
                                    op=OP.add)
                # u2 = p2 + sigmap*dw(ubar) + (sigmap/tau)*msum2~
                if C > 1:
                    V.tensor_tensor(u2[:, :R, 0:C - 1],
                                    ubar[:, alo:ahi, 1:C],
                                    ubar[:, alo:ahi, 0:C - 1], op=OP.subtract)
                V.scalar_tensor_tensor(u2[:, :R, C - 1:C],
                                       ubar[:, alo:ahi, C - 1:C],
                                       wm[:, 1:2], wsu[:, alo:ahi].unsqueeze(2),
                                       op0=OP.mult, op1=OP.add)
                if it > 0:
                    V.tensor_tensor(u2[:, :R], u2[:, :R], msum2[:, alo:ahi],
                                    op=OP.add)
                V.tensor_scalar_mul(u2[:, :R], u2[:, :R], sigmap)
                if it > 0:
                    V.tensor_tensor(u2[:, :R], u2[:, :R], p2[:, alo:ahi],
                                    op=OP.add)
                # --- cubic setup (square/relu: any table set) ---
                q2 = atile("q2")
                S.activation(q2[:, :R], u1[:, :R], AF.Square)
                S.activation(tm[:, :R], u2[:, :R], AF.Square)
                V.tensor_tensor(q2[:, :R], q2[:, :R], tm[:, :R], op=OP.add)
                bv = atile("tm")
                V.tensor_scalar_mul(bv[:, :R], q2[:, :R], 0.25)
                V.tensor_tensor(bv[:, :R], bv[:, :R], ld2[:, alo:ahi],
                                op=OP.subtract)
                msk = atile("msk", U16)
                bq = atile("bq")
                if it == 0:
                    V.tensor_scalar(msk[:, :R], bv[:, :R], 0.0, None,
                                    op0=OP.is_gt)
                    V.tensor_scalar(bq[:, :R], ld2[:, alo:ahi], -1.0 / 3.0,
                                    2.0 / 3.0, op0=OP.mult, op1=OP.add)
                else:
                    V.tensor_tensor(msk[:, :R], u3[:, :R], bv[:, :R],
                                    op=OP.is_lt)
                    V.tensor_tensor(bq[:, :R], u3[:, :R], ld2[:, alo:ahi],
                                    op=OP.add)
                    V.tensor_scalar(bq[:, :R], bq[:, :R], -1.0 / 3.0,
                                    2.0 / 3.0, op0=OP.mult, op1=OP.add)
                b3 = atile("b3")
                S.activation(b3[:, :R], bq[:, :R], AF.Square)
                V.tensor_tensor(b3[:, :R], b3[:, :R], bq[:, :R], op=OP.mult)
                dd = atile("dd")
                V.tensor_scalar_mul(dd[:, :R], q2[:, :R], 0.25)
                V.tensor_tensor(dd[:, :R], dd[:, :R], b3[:, :R], op=OP.add)
                dneg = atile("dneg", U16)
                V.tensor_scalar(dneg[:, :R], dd[:, :R], 0.0, None, op0=OP.is_lt)

                # --- ln/exp batch (no Sqrt anywhere: one act table set) ---
                lq = atile("lq")
                S.activation(lq[:, :R], q2[:, :R], AF.Ln)
                norm = atile("norm")  # = 0.5*sqrt(q2)
                S.activation(norm[:, :R], lq[:, :R], AF.Exp, scale=0.5,
                             bias=ln_half)
                rq = atile("rq")
                S.activation(rq[:, :R], lq[:, :R], AF.Exp, scale=-0.5,
                             bias=ln_two)
                sqd = atile("sqd")
                V.tensor_scalar(sqd[:, :R], dd[:, :R], 0.0, None, op0=OP.max)
                S.activation(sqd[:, :R], sqd[:, :R], AF.Ln)
                S.activation(sqd[:, :R], sqd[:, :R], AF.Exp, scale=0.5)
                lnb = atile("lnb")
                S.activation(lnb[:, :R], bq[:, :R], AF.Ln, scale=-1.0)
                sb2 = atile("b3")  # 2*sqrt(-bq) = exp(0.5*lnb + ln2)
                S.activation(sb2[:, :R], lnb[:, :R], AF.Exp, scale=0.5,
                             bias=ln_two)
                aa = atile("dd")  # reuse dd
                V.tensor_tensor(aa[:, :R], norm[:, :R], sqd[:, :R], op=OP.add)
                lt = atile("sqd")  # reuse sqd
                S.activation(lt[:, :R], aa[:, :R], AF.Ln)
                cc = atile("cc")
                S.activation(cc[:, :R], lt[:, :R], AF.Exp, scale=1.0 / 3.0)
                rc = atile("rc")
                S.activation(rc[:, :R], lt[:, :R], AF.Exp, scale=-1.0 / 3.0)
                vv = atile("vv")
                V.tensor_tensor(vv[:, :R], bq[:, :R], rc[:, :R], op=OP.mult)
                V.tensor_tensor(vv[:, :R], cc[:, :R], vv[:, :R], op=OP.subtract)
                # ratio = clip(0.5*norm*(-bq)^{-1.5}, <=1)
                eb = atile("rc")  # reuse rc
                S.activation(eb[:, :R], lnb[:, :R], AF.Exp, scale=-1.5)
                rat = atile("rat")
                V.tensor_tensor(rat[:, :R], norm[:, :R], eb[:, :R], op=OP.mult)
                V.tensor_scalar(rat[:, :R], rat[:, :R], 1.0, None, op0=OP.min)
                # y = t^2 = exp(ln(1-r) - ln(1+r))
                l1m = atile("cc")
                S.activation(l1m[:, :R], rat[:, :R], AF.Ln, scale=-1.0, bias=1.0)
                l1p = atile("dd")
                S.activation(l1p[:, :R], rat[:, :R], AF.Ln, scale=1.0, bias=1.0)
                V.tensor_tensor(l1m[:, :R], l1m[:, :R], l1p[:, :R],
                                op=OP.subtract)
                S.activation(rat[:, :R], l1m[:, :R], AF.Exp)

                # --- cos((2/3)atan(sqrt(y))) as deg-4 poly in y (in rat) ---
                PC = (0.99981162, -0.21556342, 0.11681845, -0.03518031)
                cs3 = atile("cc")
                V.tensor_scalar(cs3[:, :R], rat[:, :R], PC[3], PC[2],
                                op0=OP.mult, op1=OP.add)
                for cof in (PC[1],):
                    V.tensor_tensor(cs3[:, :R], cs3[:, :R], rat[:, :R],
                                    op=OP.mult)
                    V.tensor_scalar(cs3[:, :R], cs3[:, :R], cof, None,
                                    op0=OP.add)
                V.tensor_tensor(rat[:, :R], cs3[:, :R], rat[:, :R],
                                op=OP.mult)
                V.tensor_scalar(rat[:, :R], rat[:, :R], PC[0], None,
                                op0=OP.add)

                # --- finish (DVE + square/copy only) ---
                V.tensor_tensor(sb2[:, :R], sb2[:, :R], rat[:, :R], op=OP.mult)
                V.copy_predicated(vv[:, :R], dneg[:, :R], sb2[:, :R])
                # scl = vv * 2/norm
                V.tensor_tensor(vv[:, :R], vv[:, :R], rq[:, :R], op=OP.mult)
                nzm = atile("dneg", U16)
                V.tensor_scalar(nzm[:, :R], q2[:, :R], 0.0, None, op0=OP.is_gt)
                V.tensor_tensor(nzm[:, :R], nzm[:, :R], msk[:, :R],
                                op=OP.logical_and)
                gu = atile("rat")
                V.tensor_tensor(gu[:, :R], vv[:, :R], u1[:, :R], op=OP.mult)
                S.activation(p1[:, alo:ahi], u1[:, :R], AF.Copy)
                V.copy_predicated(p1[:, alo:ahi], nzm[:, :R], gu[:, :R])
                V.tensor_tensor(gu[:, :R], vv[:, :R], u2[:, :R], op=OP.mult)
                S.activation(p2[:, alo:ahi], u2[:, :R], AF.Copy)
                V.copy_predicated(p2[:, alo:ahi], nzm[:, :R], gu[:, :R])
                # p3 = where(msk, 0.25*(p1n^2+p2n^2) - ld2, u3)
                S.activation(q2[:, :R], p1[:, alo:ahi], AF.Square)
                S.activation(tm[:, :R], p2[:, alo:ahi], AF.Square)
                V.tensor_tensor(q2[:, :R], q2[:, :R], tm[:, :R], op=OP.add)
                V.tensor_scalar_mul(q2[:, :R], q2[:, :R], 0.25)
                V.tensor_tensor(q2[:, :R], q2[:, :R], ld2[:, alo:ahi],
                                op=OP.subtract)
                S.activation(p3[:, alo:ahi], u3[:, :R], AF.Copy)
                V.copy_predicated(p3[:, alo:ahi], msk[:, :R], q2[:, :R])

            nc.sync.dma_start(wsp[1:P, lo:hi].unsqueeze(2),
                              p2[0:P - 1, lo:hi, C - 1:C])
            # ======== B phase: interval sums, mu update, l2proj, mu->z ====
            # (outputs only feed the next iteration's A phase: skip at the end)
            bhi_all = hi - 1 if NCORES > 1 else hi
            for (blo, bhi) in ([] if it == repeats - 1
                               else _blocks(lo, bhi_all, BB)):
                R = bhi - blo
                for (pn, sx, mux, idx) in ((p2, s2, mu2, 2), (p1, s1, mu1, 1)):
                    # z-cumsum of tau*p with leading zero column (in place)
                    zct = bt_.tile([P, BB, C, 13], F16, tag="zct",
                                   name=f"zct{idx}")
                    V.memset(zct[:, :R, :, 0:1], 0.0)
                    V.tensor_scalar_mul(zct[:, :R, :, 1:13], pn[:, blo:bhi],
                                        tau_mu)
                    V.tensor_tensor_scan(
                        flat(zct[:, :R]), flat(zmb13[:, :R]),
                        flat(zct[:, :R]), 0.0, op0=OP.mult, op1=OP.add)
                    # w = dl = tau*s - t~, t~[run k2] = ics[k2] - icz[k1]:
                    # ACT broadcast-expands ics[k2] into w (no bcast penalty),
                    # DVE does packed w = icz - w (= -t~), then w += tau*s.
                    w = btile("w")
                    for k2 in range(l):
                        S.activation(
                            w[:, :R, :, off2[k2]:off2[k2 + 1]],
                            zct[:, :R, :, k2 + 1:k2 + 2]
                            .broadcast_to([P, R, C, k2 + 1]), AF.Copy)
                    for k2 in range(l):
                        V.tensor_tensor(
                            w[:, :R, :, off2[k2]:off2[k2 + 1]],
                            zct[:, :R, :, 0:k2 + 1],
                            w[:, :R, :, off2[k2]:off2[k2 + 1]],
                            op=OP.subtract)
                    mid = bhi
                    Rm = mid - blo
                    halves = [(E, hs, ws_) for (E, hs, ws_) in
                              ((V, slice(blo, mid), slice(0, Rm)),
                               (G_, slice(mid, bhi), slice(Rm, R)))
                              if hs.stop > hs.start]
                    if it == 0:
                        # s = mu = 0: dl = w; mu' = w; m = -2w
                        for (E, hs, ws_) in halves:
                            E.tensor_copy(mux[:, hs], w[:, ws_])
                            E.tensor_scalar_mul(sx[:, hs], w[:, ws_], -2.0)
                    else:
                        ts = bt_.tile([P, BB, C, PROJ], F16, tag="csp",
                                      name=f"ts{idx}")
                        for (E, hs, ws_) in halves:
                            E.tensor_scalar_mul(ts[:, ws_], sx[:, hs], tau_mu)
                            E.tensor_tensor(w[:, ws_], w[:, ws_], ts[:, ws_],
                                            op=OP.add)
                            E.tensor_tensor(mux[:, hs], mux[:, hs],
                                            w[:, ws_], op=OP.add)
                            E.tensor_tensor(sx[:, hs], sx[:, hs],
                                            mux[:, hs], op=OP.subtract)
                            E.tensor_tensor(sx[:, hs], sx[:, hs],
                                            w[:, ws_], op=OP.subtract)
                # l2proj: s = m * nu/max(|m|, nu)   (m lives in s)
                nn = btile("w")
                tb = bt_.tile([P, BB, C, PROJ], F16, tag="csp", name="tb")
                S.activation(nn[:, :R], s1[:, blo:bhi], AF.Square)
                S.activation(tb[:, :R], s2[:, blo:bhi], AF.Square)
                V.tensor_tensor(nn[:, :R], nn[:, :R], tb[:, :R], op=OP.add)
                V.tensor_scalar(nn[:, :R], nn[:, :R], nu * nu, None,
                                op0=OP.max)
                S.activation(nn[:, :R], nn[:, :R], AF.Ln)
                S.activation(nn[:, :R], nn[:, :R], AF.Exp, scale=-0.5,
                             bias=ln_nu)
                V.tensor_tensor(s1[:, blo:bhi], s1[:, blo:bhi], nn[:, :R],
                                op=OP.mult)
                V.tensor_tensor(s2[:, blo:bhi], s2[:, blo:bhi], nn[:, :R],
                                op=OP.mult)
                # mu->z sums for next iteration's A phase
                for (mux, msum, idx) in ((mu2, msum2, 2), (mu1, msum1, 1)):
                    csp = bt_.tile([P, BB, C, PROJ], F16, tag="csp",
                                   name=f"csp{idx}")
                    V.tensor_tensor_scan(
                        flat(csp[:, :R]), flat(pmb2[:, :R]),
                        flat(mux[:, blo:bhi]), 0.0, op0=OP.mult, op1=OP.add)
                    ms = msum[:, blo:bhi]
                    for k2 in range(l - 1, -1, -1):
                        seg = csp[:, :R, :, off2[k2]:off2[k2 + 1]]
                        if k2 == l - 1:
                            V.tensor_copy(ms[:, :, :, 0:l], seg)
                        else:
                            V.tensor_tensor(ms[:, :, :, 0:k2 + 1],
                                            ms[:, :, :, 0:k2 + 1], seg,
                                            op=OP.add)

            # ======== C phase: clipping ========
            for (alo, ahi) in _blocks(lo, hi, AB):
                R = ahi - alo
                pa = atile("u1")
                pc = atile("u2")
                acc = atile("u3")
                dw = atile("q2")
                # from iter 2 on un consumes only z in [1, L-1): build the
                # divergence z-sliced (zq) as well
                zq = slice(0, L) if it <= 1 else slice(1, L - 1)
                if not (alo >= G and ahi <= SLAB - G - 1):
                    V.tensor_tensor(pa[:, :R, :, zq],
                                    p1[:, alo:ahi, :, zq],
                                    mAx[:, alo:ahi, :, zq], op=OP.mult)
                    pav = pa[:, :R, :, zq]
                else:
                    pav = p1[:, alo:ahi, :, zq]
                if not (alo - 1 >= G and ahi - 1 <= SLAB - G - 1):
                    V.tensor_tensor(pc[:, :R, :, zq],
                                    p1[:, alo - 1:ahi - 1, :, zq],
                                    mCx[:, alo - 1:ahi - 1, :, zq],
                                    op=OP.mult)
                    pcv = pc[:, :R, :, zq]
                else:
                    pcv = p1[:, alo - 1:ahi - 1, :, zq]
                V.tensor_tensor(acc[:, :R, :, zq], pav, pcv, op=OP.subtract)
                if C > 1:
                    if C > 2:
                        V.tensor_tensor(dw[:, :R, 1:C - 1, zq],
                                        p2[:, alo:ahi, 1:C - 1, zq],
                                        p2[:, alo:ahi, 0:C - 2, zq],
                                        op=OP.subtract)
                    V.scalar_tensor_tensor(dw[:, :R, C - 1:C, zq],
                                           p2[:, alo:ahi, C - 1:C, zq],
                                           wm[:, 0:1],
                                           p2[:, alo:ahi, C - 2:C - 1, zq],
                                           op0=OP.mult, op1=OP.subtract)
                    V.tensor_tensor(dw[:, :R, 0:1, zq],
                                    p2[:, alo:ahi, 0:1, zq],
                                    wsp[:, alo:ahi].unsqueeze(2)[:, :, :, zq],
                                    op=OP.subtract)
                else:
                    V.scalar_tensor_tensor(dw[:, :R, 0:1, zq],
                                           p2[:, alo:ahi, 0:1, zq],
                                           wm[:, 0:1],
                                           wsp[:, alo:ahi].unsqueeze(2)
                                           [:, :, :, zq],
                                           op0=OP.mult, op1=OP.subtract)
                V.tensor_tensor(acc[:, :R, :, zq], acc[:, :R, :, zq],
                                dw[:, :R, :, zq], op=OP.add)
                if it <= 1:
                    V.tensor_tensor(acc[:, :R, :, 0:L - 1],
                                    acc[:, :R, :, 0:L - 1],
                                    p3[:, alo:ahi, :, 0:L - 1], op=OP.add)
                    V.tensor_tensor(acc[:, :R, :, 1:L], acc[:, :R, :, 1:L],
                                    p3[:, alo:ahi, :, 0:L - 1],
                                    op=OP.subtract)
                else:
                    V.tensor_tensor(acc[:, :R, :, 1:L - 1],
                                    acc[:, :R, :, 1:L - 1],
                                    p3[:, alo:ahi, :, 1:L - 1], op=OP.add)
                    V.tensor_tensor(acc[:, :R, :, 1:L - 1],
                                    acc[:, :R, :, 1:L - 1],
                                    p3[:, alo:ahi, :, 0:L - 2],
                                    op=OP.subtract)
                # un = clip(u + tauu*acc); boundary z; ubar = 2un - u
                # from iter 2 on, z-cols 0 and L-1 of u AND ubar are
                # constant (1.0 / 0.0): update only z in [1, L-1)
                zs = slice(0, L) if it <= 1 else slice(1, L - 1)
                zw = L if it <= 1 else L - 2
                un = atile("tm")
                V.tensor_scalar_mul(un[:, :R, :, 0:zw], acc[:, :R, :, zs],
                                    tauu)
                V.tensor_tensor(un[:, :R, :, 0:zw], un[:, :R, :, 0:zw],
                                u[:, alo:ahi, :, zs], op=OP.add)
                V.tensor_scalar(un[:, :R, :, 0:zw], un[:, :R, :, 0:zw],
                                0.0, 1.0, op0=OP.max, op1=OP.min)
                if it <= 1:
                    V.memset(un[:, :R, :, 0:1], 1.0)
                    V.memset(un[:, :R, :, L - 1:L], 0.0)
                if it < repeats - 1:
                    V.tensor_scalar_mul(acc[:, :R, :, 0:zw],
                                        un[:, :R, :, 0:zw], 2.0)
                    V.tensor_tensor(ubar[:, alo:ahi, :, zs],
                                    acc[:, :R, :, 0:zw],
                                    u[:, alo:ahi, :, zs], op=OP.subtract)
                S.activation(u[:, alo:ahi, :, zs], un[:, :R, :, 0:zw],
                             AF.Copy)
                if it == repeats - 1:
                    olo, ohi = max(alo, G), min(ahi, G + ROWS)
                    if olo < ohi:
                        nc.sync.dma_start(
                            u_out.ap()[:, (olo - G) * C * L:(ohi - G) * C * L],
                            flat(u[:, olo:ohi]))

            if it < repeats - 1:
                nlo, nhi = it + 2, SLAB - 2 - it
                nablo = max(nlo - 1, 0)
                nc.sync.dma_start(wsu[0:P - 1, nablo:nhi].unsqueeze(2),
                                  ubar[1:P, nablo:nhi, 0:1])

        # (output is streamed per C-block of the last iteration)

    nc.compile()
    return nc


_cache = {}


def _get_program(lmbda, nu, repeats, l, cfg_key=None):
    key = (float(lmbda), float(nu), int(repeats), int(l))
    if key not in _cache:
        _cache[key] = build_program(float(lmbda), float(nu), int(repeats),
                                    int(l))
    return _cache[key]


def make_inputs(f, repeats, cfg=None):
    cfg = cfg or CFG
    H, W, L, NCORES, P = cfg["H"], cfg["W"], cfg["L"], cfg["NCORES"], cfg["P"]
    C = W // P
    ROWS = H // NCORES
    G = int(repeats)
    SLAB = ROWS + 2 * G
    f2 = np.asarray(f, dtype=np.float32).reshape(H, W)
    fpad = np.zeros((H + 2 * G, W), np.float32)
    fpad[G:G + H] = f2
    in_maps = []
    for k in range(NCORES):
        slab = fpad[k * ROWS: k * ROWS + SLAB]              # [SLAB, W]
        arr = slab.reshape(SLAB, P, C).transpose(1, 0, 2)   # [P, SLAB, C]
        g = np.arange(SLAB) + k * ROWS - G                  # global row ids
        mAv = ((g >= 0) & (g <= H - 2)).astype(np.float16)
        mCv = ((g >= 0) & (g <= H - 1)).astype(np.float16)
        wmv = np.ones((P, 2), np.float32)
        wmv[:, 1] = -1.0
        wmv[P - 1, :] = 0.0
        in_maps.append({
            "f_in": np.ascontiguousarray(arr.reshape(P, SLAB * C)),
            "mA_in": np.ascontiguousarray(np.broadcast_to(mAv, (P, SLAB))),
            "mC_in": np.ascontiguousarray(np.broadcast_to(mCv, (P, SLAB))),
            "wm_in": wmv,
        })
    return in_maps


def assemble_output(results, repeats, cfg=None):
    cfg = cfg or CFG
    H, W, L, NCORES, P = cfg["H"], cfg["W"], cfg["L"], cfg["NCORES"], cfg["P"]
    C = W // P
    ROWS = H // NCORES
    out = np.empty((H, W, 1, L), np.float32)
    for k in range(NCORES):
        o = results[k]["u_out"].astype(np.float32).reshape(P, ROWS, C, L)
        out[k * ROWS:(k + 1) * ROWS, :, 0, :] = (
            o.transpose(1, 0, 2, 3).reshape(ROWS, W, L))
    return out


def kernel(f, lmbda, nu, repeats, l):
    l = int(l)
    repeats = int(repeats)
    cfg = dict(CFG)
    cfg["L"] = l
    key = (float(lmbda), float(nu), repeats, l)
    if key not in _cache:
        _cache[key] = build_program(float(lmbda), float(nu), repeats, l,
                                    cfg=cfg)
    nc = _cache[key]
    in_maps = make_inputs(np.asarray(f, np.float32), repeats, cfg=cfg)
    res = run_bass_kernel_spmd(nc, in_maps,
                               core_ids=list(range(cfg["NCORES"])))
    return assemble_output(res.results, repeats, cfg=cfg)
